# revision 1
# baseline (speedup 1.0000x reference)
"""Trainium2 Bass kernel for nn_Block_343597384085.

Model (per batch b):
  c        = silu(causal_depthwise_conv(x, K=4) + conv_b)
  out_gate = silu(x @ gate_w + gate_b)
  v = ctx = out = c
  for i in 0..3:
      cn      = rmsnorm(ctx) * rms_w[i]
      alphas  = sigmoid(cn @ alpha_w[i] + alpha_b[i])
      betas   = silu(cn @ beta_w[i] + beta_b[i])
      ws      = sqrt(clip(1 - alphas^2, 1e-6))
      fetched = assoc_scan(h_t = a_t h_{t-1} + v_t) over (v*betas*ws, alphas)
      ctx     = ctx + silu(fetched @ ctx_w[i] + ctx_b[i])
      out     = out + fetched
  out = rmsnorm(out * out_gate) * fin_rms_w
  y   = silu(out @ fin_w + fin_b)

Sharding: 8 cores = (batch, seq-half). Each core: 1024 tokens x D=1024,
feature-major SBUF layout [D-block(128 part), tokens(free)].
The scan's cross-half carry moves via a per-iteration pair AllGather (4KB);
each core then computes fetched = h_local + cumprod(alpha)*carry, with
carry masked to 0 on even (first-half) cores so the program is uniform.
All matmuls run in float32r (full PE speed, ~1e-4 relative error).
"""
import numpy as np

import concourse.bass as bass
import concourse.bacc as bacc
import concourse.mybir as mybir
import concourse.tile as tile
from concourse import bass_utils, masks

B, S, D, N, K = 4, 2048, 1024, 4, 4
EPS = 1e-6
P = 128                 # partitions per feature block
NB = D // P             # 8 feature blocks
T = S // 2              # tokens per core
SUB = 512               # matmul moving-dim tile (one fp32 PSUM bank)
NS = T // SUB           # sub-tiles per core
F32 = mybir.dt.float32
F32R = mybir.dt.float32r
OP = mybir.AluOpType
AF = mybir.ActivationFunctionType

_CACHE = {}
ABL = set()          # timing-only ablation flags (test harness use)


def _build(reps=1, no_cc=False):
    nc = bacc.Bacc("TRN2", target_bir_lowering=False, debug=False, num_devices=8)

    # per-core inputs
    xm_d = nc.dram_tensor("xm", [T, D], F32, kind="ExternalInput")
    xhalo_d = nc.dram_tensor("xhalo", [P, NB * (K - 1)], F32, kind="ExternalInput")
    fbrow_d = nc.dram_tensor("fbrow", [P, D], F32, kind="ExternalInput")
    mask_d = nc.dram_tensor("mask", [P, 1], F32, kind="ExternalInput")
    # packed per-partition aux: [P, NB] / [P, N*NB] with col = i*NB + nb
    cwp_d = nc.dram_tensor("cwp", [P, NB * K], F32, kind="ExternalInput")
    cbp_d = nc.dram_tensor("cbp", [P, NB], F32, kind="ExternalInput")
    gbp_d = nc.dram_tensor("gbp", [P, NB], F32, kind="ExternalInput")
    rwp_d = nc.dram_tensor("rwp", [P, N * NB], F32, kind="ExternalInput")
    abp_d = nc.dram_tensor("abp", [P, N * NB], F32, kind="ExternalInput")
    bbp_d = nc.dram_tensor("bbp", [P, N * NB], F32, kind="ExternalInput")
    ctbp_d = nc.dram_tensor("ctbp", [P, N * NB], F32, kind="ExternalInput")
    frwp_d = nc.dram_tensor("frwp", [P, NB], F32, kind="ExternalInput")
    fbp_d = nc.dram_tensor("fbp", [P, NB], F32, kind="ExternalInput")
    gw_d = nc.dram_tensor("gate_w", [D, D], F32, kind="ExternalInput")
    aw_d = nc.dram_tensor("alpha_w", [N, D, D], F32, kind="ExternalInput")
    bw_d = nc.dram_tensor("beta_w", [N, D, D], F32, kind="ExternalInput")
    cw_d = nc.dram_tensor("ctx_w", [N, D, D], F32, kind="ExternalInput")
    fw_d = nc.dram_tensor("fin_w", [D, D], F32, kind="ExternalInput")
    y_d = nc.dram_tensor("y", [T, D], F32, kind="ExternalOutput")

    with tile.TileContext(nc) as tc:
        _emit(nc, tc, locals(), reps=reps, no_cc=no_cc)
    nc.compile()
    return nc


def _emit(nc, tc, t, reps=1, no_cc=False):
    xm_d = t["xm_d"]; xhalo_d = t["xhalo_d"]; fbrow_d = t["fbrow_d"]
    mask_d = t["mask_d"]; cwp_d = t["cwp_d"]
    cbp_d = t["cbp_d"]; gbp_d = t["gbp_d"]; rwp_d = t["rwp_d"]
    abp_d = t["abp_d"]; bbp_d = t["bbp_d"]; ctbp_d = t["ctbp_d"]
    frwp_d = t["frwp_d"]; fbp_d = t["fbp_d"]; gw_d = t["gw_d"]
    aw_d = t["aw_d"]; bw_d = t["bw_d"]; cw_d = t["cw_d"]; fw_d = t["fw_d"]
    y_d = t["y_d"]

    import contextlib
    with contextlib.ExitStack() as est:
        aux = est.enter_context(tc.tile_pool(name="aux", bufs=1))
        state = est.enter_context(tc.tile_pool(name="state", bufs=1))
        wp = est.enter_context(tc.tile_pool(name="wp", bufs=2))     # weight slabs
        tmp = est.enter_context(tc.tile_pool(name="tmp", bufs=4))   # [P,SUB] f32 transients
        cfp = est.enter_context(tc.tile_pool(name="cf", bufs=1))    # cn/fetched/fo (f32r)
        alp = est.enter_context(tc.tile_pool(name="alp", bufs=1))   # alphas (+reuse)
        sip = est.enter_context(tc.tile_pool(name="sip", bufs=1))   # scan_in/h (+reuse)
        vwp = est.enter_context(tc.tile_pool(name="vwp", bufs=1))   # v stream
        sqr = est.enter_context(tc.tile_pool(name="sqr", bufs=2))   # [P,SUB] f32r transients
        mmp = est.enter_context(tc.tile_pool(name="mmp", bufs=4, space="PSUM"))
        ssp = est.enter_context(tc.tile_pool(name="ssp", bufs=2, space="PSUM"))
        bcp = est.enter_context(tc.tile_pool(name="bcp", bufs=2, space="PSUM"))
        dram = est.enter_context(tc.tile_pool(name="dram", bufs=1, space="DRAM"))

        # ---- aux constants ----
        def aux_load(name, dram_t, shape):
            tl = aux.tile(shape, F32, name=name)
            nc.sync.dma_start(tl[:], dram_t[:])
            return tl
        mask = aux_load("mask", mask_d, [P, 1])
        cwp = aux_load("cwp", cwp_d, [P, NB * K])
        cbp = aux_load("cbp", cbp_d, [P, NB])
        gbp = aux_load("gbp", gbp_d, [P, NB])
        rwp = aux_load("rwp", rwp_d, [P, N * NB])
        abp = aux_load("abp", abp_d, [P, N * NB])
        bbp = aux_load("bbp", bbp_d, [P, N * NB])
        ctbp = aux_load("ctbp", ctbp_d, [P, N * NB])
        frwp = aux_load("frwp", frwp_d, [P, NB])
        fbp = aux_load("fbp", fbp_d, [P, NB])
        ones_f = aux.tile([P, 1], F32)
        nc.vector.memset(ones_f[:], 1.0)
        ones_r = aux.tile([P, 1], F32R)
        nc.vector.tensor_copy(ones_r[:], ones_f[:])
        ones1_f = aux.tile([1, P], F32)
        nc.vector.memset(ones1_f[:], 1.0)
        ones1_r = aux.tile([1, P], F32R)
        nc.vector.tensor_copy(ones1_r[:], ones1_f[:])
        eps_t = aux.tile([P, 1], F32)
        nc.vector.memset(eps_t[:], EPS)
        xhalo = aux_load("xhalo", xhalo_d, [P, NB * (K - 1)])
        fbrow = aux_load("fbrow", fbrow_d, [P, D])
        ident = aux.tile([P, P], F32)
        masks.make_identity(nc, ident[:])

        # ---- DRAM scratch ----
        v_s = [dram.tile([P, T], F32, name=f"v_s{nb}") for nb in range(NB)]
        og_s = [dram.tile([P, T], F32, name=f"og_s{nb}") for nb in range(NB)]
        oacc = [dram.tile([P, T], F32, name=f"oacc{nb}") for nb in range(NB)]

        # persistent ctx
        ctxb = [state.tile([P, T], F32, name=f"ctx{nb}") for nb in range(NB)]

        def rms_inv(src, sl, tag_suffix):
            """1/sqrt(mean_d(src^2) + eps) broadcast to [P, SUB]."""
            ssps = ssp.tile([1, SUB], F32, tag="ss", name=f"ss{tag_suffix}")
            for nb in range(NB):
                sq = sqr.tile([P, SUB], F32R, tag="sq", name=f"sq{tag_suffix}_{nb}")
                nc.scalar.activation(sq[:], src[nb][:, sl], AF.Square)
                nc.tensor.matmul(ssps[:], ones_r[:], sq[:],
                                 start=(nb == 0), stop=(nb == NB - 1))
            ssr = sqr.tile([1, SUB], F32R, tag="sq", name=f"ssr{tag_suffix}")
            nc.scalar.copy(ssr[:], ssps[:])
            bc = bcp.tile([P, SUB], F32, tag="bc", name=f"bc{tag_suffix}")
            nc.tensor.matmul(bc[:], ones1_r[:], ssr[:], start=True, stop=True)
            sd = tmp.tile([P, SUB], F32, tag="tmp", name=f"sd{tag_suffix}")
            nc.scalar.activation(sd[:], bc[:], AF.Sqrt, bias=eps_t[:, 0:1],
                                 scale=1.0 / D)
            inv = tmp.tile([P, SUB], F32, tag="tmp", name=f"inv{tag_suffix}")
            nc.vector.reciprocal(inv[:], sd[:])
            return inv

        def one_pass(rep):
            # ---- phase 0: x load + PE transpose to [D, T], conv, gate ----
            xT = []
            for nb in range(NB):
                xt = sip.tile([P, T + K - 1], F32R, tag=f"sin{nb}",
                              name=f"r{rep}_xT{nb}")
                nc.vector.tensor_copy(xt[:, 0:K - 1],
                                      xhalo[:, nb * (K - 1):(nb + 1) * (K - 1)])
                xT.append(xt)
            for tb in range(NB):
                slab = alp.tile([P, D], F32, tag=f"al{tb % 3}",
                                name=f"r{rep}_slab{tb}")
                nc.sync.dma_start(slab[:], xm_d[tb * P:(tb + 1) * P, :])
                for nb in range(NB):
                    pst = mmp.tile([P, P], F32, tag="mm",
                                   name=f"r{rep}_pst{tb}_{nb}")
                    nc.tensor.transpose(pst[:], slab[:, nb * P:(nb + 1) * P],
                                        ident[:])
                    nc.scalar.copy(
                        xT[nb][:, K - 1 + tb * P:K - 1 + (tb + 1) * P], pst[:])

            for nb in range(NB):
                xf = xT[nb].bitcast(F32)
                cacc = alp.tile([P, T], F32, tag=f"al{nb}", name=f"r{rep}_cacc{nb}")
                nc.vector.tensor_scalar(
                    cacc[:], xf[:, 0:T], cwp[:, nb * K:nb * K + 1], None, OP.mult)
                for k in range(1, K):
                    nc.vector.scalar_tensor_tensor(
                        cacc[:], xf[:, k:k + T], cwp[:, nb * K + k:nb * K + k + 1],
                        cacc[:], OP.mult, OP.add)
                vsb = cfp.tile([P, T], F32, tag=f"cf{nb}", name=f"r{rep}_vsb{nb}")
                nc.scalar.activation(vsb[:], cacc[:], AF.Silu,
                                     bias=cbp[:, nb:nb + 1])
                nc.sync.dma_start(v_s[nb][:], vsb[:])
                nc.sync.dma_start(oacc[nb][:], vsb[:])
                nc.vector.tensor_copy(ctxb[nb][:], vsb[:])

            gw = []
            for k in range(NB):
                gwk = wp.tile([P, D], F32R, tag=f"w{k}", name=f"r{rep}_gw{k}",
                              bufs=2 if k < 4 else 1)
                nc.sync.dma_start(gwk[:], gw_d[k * P:(k + 1) * P, :].bitcast(F32R))
                gw.append(gwk)
            for m in range(NB):
                for s in range(NS):
                    ps = mmp.tile([P, SUB], F32, tag="mm", name=f"r{rep}_psg{m}_{s}")
                    for k in range(NB):
                        nc.tensor.matmul(
                            ps[:], gw[k][:, m * P:(m + 1) * P],
                            xT[k][:, K - 1 + s * SUB:K - 1 + (s + 1) * SUB],
                            start=(k == 0), stop=(k == NB - 1))
                    ogt = tmp.tile([P, SUB], F32, tag="tmp", name=f"r{rep}_og{m}_{s}")
                    nc.scalar.activation(ogt[:], ps[:], AF.Silu,
                                         bias=gbp[:, m:m + 1])
                    nc.sync.dma_start(og_s[m][:, s * SUB:(s + 1) * SUB], ogt[:])

            # ---- iterations ----
            for i in range(N):
                # R: cn = rmsnorm(ctx) * rms_w[i]
                cn = [cfp.tile([P, T], F32R, tag=f"cf{nb}", name=f"r{rep}_cn{i}_{nb}")
                      for nb in range(NB)]
                for s in range(NS):
                    sl = slice(s * SUB, (s + 1) * SUB)
                    inv = rms_inv(ctxb, sl, f"r{i}_{s}")
                    for nb in range(NB):
                        nc.vector.scalar_tensor_tensor(
                            cn[nb][:, sl], ctxb[nb][:, sl],
                            rwp[:, i * NB + nb:i * NB + nb + 1], inv[:],
                            OP.mult, OP.mult)

                # A: alphas = sigmoid(cn @ alpha_w[i] + alpha_b[i])
                wa = []
                for k in range(NB):
                    wak = wp.tile([P, D], F32R, tag=f"w{k}", name=f"r{rep}_wa{i}_{k}",
                                  bufs=2 if k < 4 else 1)
                    nc.sync.dma_start(
                        wak[:], aw_d[i, k * P:(k + 1) * P, :].bitcast(F32R))
                    wa.append(wak)
                alphas = [alp.tile([P, T], F32, tag=f"al{nb}", name=f"r{rep}_alphas{i}_{nb}")
                          for nb in range(NB)]
                for m in range(NB):
                    for s in range(NS):
                        sl = slice(s * SUB, (s + 1) * SUB)
                        ps = mmp.tile([P, SUB], F32, tag="mm", name=f"r{rep}_psa{i}_{m}_{s}")
                        for k in range(NB):
                            nc.tensor.matmul(ps[:], wa[k][:, m * P:(m + 1) * P],
                                             cn[k][:, sl],
                                             start=(k == 0), stop=(k == NB - 1))
                        nc.scalar.activation(alphas[m][:, sl], ps[:], AF.Sigmoid,
                                             bias=abp[:, i * NB + m:i * NB + m + 1])

                # B: scan_in = v * silu(cn@beta_w+b) * sqrt(1-alphas^2);
                #    then in-place h-scan per block, carry = last column
                wb = []
                for k in range(NB):
                    wbk = wp.tile([P, D], F32R, tag=f"w{k}", name=f"r{rep}_wb{i}_{k}",
                                  bufs=2 if k < 4 else 1)
                    nc.sync.dma_start(
                        wbk[:], bw_d[i, k * P:(k + 1) * P, :].bitcast(F32R))
                    wb.append(wbk)
                sin = [sip.tile([P, T], F32, tag=f"sin{nb}", name=f"r{rep}_sin{i}_{nb}")
                       for nb in range(NB)]
                carries = aux.tile([P, NB], F32, name=f"r{rep}_carries{i}")
                # ws pre-pass, batched so ACT stays in the sqrt table set:
                # sin[m] = sqrt(1 - alphas^2)
                sqf = AF.Copy if "nosq" in ABL else AF.Square
                rtf = AF.Copy if "nosq" in ABL else AF.Sqrt
                for m in range(NB):
                    for s in range(NS):
                        sl = slice(s * SUB, (s + 1) * SUB)
                        asq = tmp.tile([P, SUB], F32, tag="tmp", name=f"r{rep}_asq{i}_{m}_{s}")
                        nc.scalar.activation(asq[:], alphas[m][:, sl], sqf)
                        nc.vector.tensor_scalar(asq[:], asq[:], -1.0, 1.0,
                                                OP.mult, OP.add)
                        nc.scalar.activation(sin[m][:, sl], asq[:], rtf)
                for m in range(NB):
                    vw = vwp.tile([P, T], F32, tag="vw", name=f"r{rep}_vw{i}_{m}")
                    nc.sync.dma_start(vw[:], v_s[m][:])
                    for s in range(NS):
                        sl = slice(s * SUB, (s + 1) * SUB)
                        ps = mmp.tile([P, SUB], F32, tag="mm", name=f"r{rep}_psb{i}_{m}_{s}")
                        for k in range(NB):
                            nc.tensor.matmul(ps[:], wb[k][:, m * P:(m + 1) * P],
                                             cn[k][:, sl],
                                             start=(k == 0), stop=(k == NB - 1))
                        bet = tmp.tile([P, SUB], F32, tag="tmp", name=f"r{rep}_bet{i}_{m}_{s}")
                        nc.scalar.activation(bet[:], ps[:], AF.Silu,
                                             bias=bbp[:, i * NB + m:i * NB + m + 1])
                        # scan_in = (betas * ws) * v, in place over sin
                        nc.vector.tensor_tensor(sin[m][:, sl], bet[:],
                                                sin[m][:, sl], OP.mult)
                        nc.vector.tensor_tensor(sin[m][:, sl], sin[m][:, sl],
                                                vw[:, sl], OP.mult)
                    # local scan (initial 0), in place; carry = last column
                    if "noscan" in ABL:
                        nc.vector.tensor_copy(sin[m][:], alphas[m][:])
                    else:
                        nc.vector.tensor_tensor_scan(sin[m][:], alphas[m][:],
                                                     sin[m][:], 0.0,
                                                     OP.mult, OP.add)
                    nc.vector.tensor_copy(carries[:, m:m + 1], sin[m][:, T - 1:T])

                # carry exchange: pair AllGather; c_eff = mask * even-partner carry
                cin = dram.tile([D], F32, name=f"r{rep}_cin{i}")
                cout = dram.tile([2, D], F32, name=f"r{rep}_cout{i}")
                if "fastcarry" in ABL:
                    nc.sync.dma_start(cin[:].rearrange("(p nb) -> p nb", p=P),
                                      carries[:])
                else:
                    nc.sync.dma_start(cin[:].rearrange("(nb p) -> p nb", p=P),
                                      carries[:])
                if no_cc:
                    nc.sync.dma_start(cout[0:1, :],
                                      cin[:].rearrange("(a b) -> a b", a=1))
                    nc.sync.dma_start(cout[1:2, :],
                                      cin[:].rearrange("(a b) -> a b", a=1))
                else:
                    nc.gpsimd.collective_compute(
                        "AllGather", OP.bypass,
                        replica_groups=[[0, 1], [2, 3], [4, 5], [6, 7]],
                        ins=[cin.opt()], outs=[cout.opt()])
                gsb = aux.tile([P, NB], F32, name=f"r{rep}_gsb{i}")
                if "fastcarry" in ABL:
                    nc.sync.dma_start(
                        gsb[:], cout[0:1, :].rearrange("a (p nb) -> (a p) nb", p=P))
                else:
                    nc.sync.dma_start(
                        gsb[:], cout[0:1, :].rearrange("a (nb p) -> (a p) nb", p=P))
                ceff = aux.tile([P, NB], F32, name=f"r{rep}_ceff{i}")
                nc.vector.tensor_scalar(ceff[:], gsb[:], mask[:, 0:1], None, OP.mult)

                # correction: g = cumprod(alphas)*c (in place over alphas),
                # fetched = h_local + g  (f32r, into the freed cn slots)
                fetched = []
                for nb in range(NB):
                    if "noscan" in ABL:
                        nc.vector.tensor_copy(alphas[nb][:], sin[nb][:])
                    else:
                        nc.vector.tensor_tensor_scan(
                            alphas[nb][:], alphas[nb][:], alphas[nb][:],
                            ceff[:, nb:nb + 1], OP.mult, OP.bypass)
                    fe = cfp.tile([P, T], F32R, tag=f"cf{nb}", name=f"r{rep}_fe{i}_{nb}")
                    nc.vector.tensor_tensor(fe[:], sin[nb][:], alphas[nb][:], OP.add)
                    fetched.append(fe)
                    # out += fetched (DMA accumulate into DRAM)
                    if "noaccum" in ABL:
                        nc.sync.dma_start(oacc[nb][:], fe.bitcast(F32)[:])
                    else:
                        nc.gpsimd.dma_start(oacc[nb][:], fe.bitcast(F32)[:],
                                            accum_op=OP.add)

                # C: ctx += silu(fetched @ ctx_w[i] + ctx_b[i])
                wc = []
                for k in range(NB):
                    wck = wp.tile([P, D], F32R, tag=f"w{k}", name=f"r{rep}_wc{i}_{k}",
                                  bufs=2 if k < 4 else 1)
                    nc.sync.dma_start(
                        wck[:], cw_d[i, k * P:(k + 1) * P, :].bitcast(F32R))
                    wc.append(wck)
                for m in range(NB):
                    for s in range(NS):
                        sl = slice(s * SUB, (s + 1) * SUB)
                        ps = mmp.tile([P, SUB], F32, tag="mm", name=f"r{rep}_psc{i}_{m}_{s}")
                        for k in range(NB):
                            nc.tensor.matmul(ps[:], wc[k][:, m * P:(m + 1) * P],
                                             fetched[k][:, sl],
                                             start=(k == 0), stop=(k == NB - 1))
                        cu = tmp.tile([P, SUB], F32, tag="tmp", name=f"r{rep}_cu{i}_{m}_{s}")
                        nc.scalar.activation(cu[:], ps[:], AF.Silu,
                                             bias=ctbp[:, i * NB + m:i * NB + m + 1])
                        eng = nc.vector if "nogps" in ABL else nc.gpsimd
                        eng.tensor_tensor(ctxb[m][:, sl], ctxb[m][:, sl],
                                          cu[:], OP.add)

            # ---- final: y = silu(rmsnorm(out*gate)*fin_rms_w @ fin_w + fin_b)
            po = [sip.tile([P, T], F32, tag=f"sin{nb}", name=f"r{rep}_po{nb}")
                  for nb in range(NB)]
            for nb in range(NB):
                ogl = vwp.tile([P, T], F32, tag="vw", name=f"r{rep}_ogl{nb}")
                nc.sync.dma_start(ogl[:], og_s[nb][:])
                oal = alp.tile([P, T], F32, tag=f"al{nb}", name=f"r{rep}_oal{nb}")
                nc.sync.dma_start(oal[:], oacc[nb][:])
                nc.vector.tensor_tensor(po[nb][:], oal[:], ogl[:], OP.mult)
            fo = [cfp.tile([P, T], F32R, tag=f"cf{nb}", name=f"r{rep}_fo{nb}")
                  for nb in range(NB)]
            for s in range(NS):
                sl = slice(s * SUB, (s + 1) * SUB)
                inv = rms_inv(po, sl, f"f{s}")
                for nb in range(NB):
                    nc.vector.scalar_tensor_tensor(
                        fo[nb][:, sl], po[nb][:, sl], frwp[:, nb:nb + 1], inv[:],
                        OP.mult, OP.mult)
            fw = []
            for k in range(NB):
                fwk = wp.tile([P, D], F32R, tag=f"w{k}", name=f"r{rep}_fw{k}",
                              bufs=2 if k < 4 else 1)
                nc.sync.dma_start(fwk[:], fw_d[k * P:(k + 1) * P, :].bitcast(F32R))
                fw.append(fwk)
            for tb in range(NB):
                for do in range(NS):
                    ps = mmp.tile([P, SUB], F32, tag="mm", name=f"r{rep}_psf{tb}_{do}")
                    for k in range(NB):
                        nc.tensor.matmul(ps[:], fo[k][:, tb * P:(tb + 1) * P],
                                         fw[k][:, do * SUB:(do + 1) * SUB],
                                         start=(k == 0), stop=(k == NB - 1))
                    yt = tmp.tile([P, SUB], F32, tag="tmp", name=f"r{rep}_yt{tb}_{do}")
                    nc.vector.tensor_tensor(yt[:], ps[:],
                                            fbrow[:, do * SUB:(do + 1) * SUB],
                                            OP.add)
                    nc.scalar.activation(yt[:], yt[:], AF.Silu)
                    nc.sync.dma_start(
                        y_d[tb * P:(tb + 1) * P, do * SUB:(do + 1) * SUB], yt[:])


        for rep in range(reps):
            one_pass(rep)


def _prep_in_maps(inputs):
    x = np.asarray(inputs["x"], np.float32)
    conv_w = np.asarray(inputs["conv_w"], np.float32)
    conv_b = np.asarray(inputs["conv_b"], np.float32)
    gate_w = np.asarray(inputs["gate_w"], np.float32)
    gate_b = np.asarray(inputs["gate_b"], np.float32)
    rms_w = np.asarray(inputs["rms_w"], np.float32)
    alpha_w = np.asarray(inputs["alpha_w"], np.float32)
    alpha_b = np.asarray(inputs["alpha_b"], np.float32)
    beta_w = np.asarray(inputs["beta_w"], np.float32)
    beta_b = np.asarray(inputs["beta_b"], np.float32)
    ctx_w = np.asarray(inputs["ctx_w"], np.float32)
    ctx_b = np.asarray(inputs["ctx_b"], np.float32)
    fin_rms_w = np.asarray(inputs["fin_rms_w"], np.float32)
    fin_w = np.asarray(inputs["fin_w"], np.float32)
    fin_b = np.asarray(inputs["fin_b"], np.float32)

    def pack1(a):       # [D] -> [P, NB]
        return np.ascontiguousarray(a.reshape(NB, P).T)

    def packN(a):       # [N, D] -> [P, N*NB]
        return np.ascontiguousarray(
            a.reshape(N, NB, P).transpose(2, 0, 1).reshape(P, N * NB))

    cwp = np.ascontiguousarray(
        conv_w.T.reshape(NB, P, K).transpose(1, 0, 2).reshape(P, NB * K))
    shared = dict(
        cwp=cwp, cbp=pack1(conv_b), gbp=pack1(gate_b),
        rwp=packN(rms_w), abp=packN(alpha_b), bbp=packN(beta_b),
        ctbp=packN(ctx_b), frwp=pack1(fin_rms_w), fbp=pack1(fin_b),
        gate_w=np.ascontiguousarray(gate_w),
        alpha_w=np.ascontiguousarray(alpha_w),
        beta_w=np.ascontiguousarray(beta_w),
        ctx_w=np.ascontiguousarray(ctx_w),
        fin_w=np.ascontiguousarray(fin_w),
    )
    shared["fbrow"] = np.ascontiguousarray(
        np.broadcast_to(fin_b[None, :], (P, D)))
    in_maps = []
    for c in range(8):
        b, h = c // 2, c % 2
        t0 = h * T
        m = dict(shared)
        m["xm"] = np.ascontiguousarray(x[b, t0:t0 + T])
        if h == 0:
            m["xhalo"] = np.zeros((P, NB * (K - 1)), np.float32)
        else:
            halo = x[b, t0 - (K - 1):t0, :]          # [K-1, D]
            m["xhalo"] = np.ascontiguousarray(
                halo.T.reshape(NB, P, K - 1).transpose(1, 0, 2)
                .reshape(P, NB * (K - 1)))
        m["mask"] = np.full((P, 1), float(h), np.float32)
        in_maps.append(m)
    return in_maps


def kernel(**inputs) -> np.ndarray:
    if "nc" not in _CACHE:
        _CACHE["nc"] = _build()
    nc = _CACHE["nc"]
    in_maps = _prep_in_maps(inputs)
    res = bass_utils.run_bass_kernel_spmd(nc, in_maps, core_ids=list(range(8)))
    y = np.empty((B, S, D), np.float32)
    for c in range(8):
        b, h = c // 2, c % 2
        y[b, h * T:(h + 1) * T] = res.results[c]["y"]
    return y



# revision 4
# speedup vs baseline: 1.2156x; 1.2156x over previous
"""Trainium2 Bass kernel for nn_Block_343597384085 (mixed precision).

Per-core (8 cores = batch x seq-half), feature-major [D-block(128p), tokens].

Dtype plan (validated in numpy, end-to-end rel err ~1.2e-2 vs 2e-2 gate):
  alpha/ctx matmuls : fp8 e4m3 DoubleRow (2 contract rows/cycle)
  beta/gate/fin mms : bf16 weights + bf16 moving
  intermediates     : bf16 (v, cn, betas, ws, sin/h, fetched, ctx, out, og)
  alphas + scan     : f32 (EMA amplifies post-sigmoid error by 1/(1-a))
Quant scales (fixed, host-side): cn x32, alpha_w x512 ; ctx path: the ws
pre-scale trick puts x4 on sin/h/fetched (ws' = sqrt(16-16a^2)) so fetched
is already fp8-ranged; ctx_w x32768. Dequants fold into ACT scale at PSUM
evacuation. out accumulates 4x(v + sum fe); the final rmsnorm cancels the
global 4x exactly.

Schedule: cumprod runs pre-AllGather (init=1) and the carry applies via one
fused scalar_tensor_tensor; v/gate-x prefetched; the gate matmul is deferred
into the per-iteration AllGather windows as PE filler; the carry exchange
is split into two 2KB pair-AllGathers so the first hides under the second
half of the beta matmul; rms reduces bf16 squares on the PE and takes
1/sqrt on a [1,512] row (no [P,512] reciprocals).
"""
import numpy as np
import ml_dtypes

import concourse.bass as bass
import concourse.bacc as bacc
import concourse.mybir as mybir
import concourse.tile as tile
from concourse import bass_utils, masks

B, S, D, N, K = 4, 2048, 1024, 4, 4
EPS = 1e-6
P = 128
NB = D // P             # 8 feature blocks
K2 = NB // 2            # 4 fp8 pair-blocks
T = S // 2              # tokens per core
SUB = 512
NS = T // SUB
F32 = mybir.dt.float32
F32R = mybir.dt.float32r
BF16 = mybir.dt.bfloat16
F8 = mybir.dt.float8e4
OP = mybir.AluOpType
AF = mybir.ActivationFunctionType
PM = mybir.MatmulPerfMode

S_CN = 8.0
S_AW = 512.0
S_CW = 32768.0
S_FE = 4.0              # via ws' = sqrt(16 - 16 a^2)
DQ_A = 1.0 / (S_CN * S_AW)
DQ_C = 1.0 / (S_FE * S_CW)

E4NP = ml_dtypes.float8_e4m3
BFNP = ml_dtypes.bfloat16

_CACHE = {}


def _build(reps=1, no_cc=False):
    nc = bacc.Bacc("TRN2", target_bir_lowering=False, debug=False, num_devices=8)

    t = {}
    t["xtb_d"] = nc.dram_tensor("xtb", [NB, P, K - 1 + T], BF16,
                                kind="ExternalInput")
    t["mask_d"] = nc.dram_tensor("mask", [P, 1], F32, kind="ExternalInput")
    t["cwp_d"] = nc.dram_tensor("cwp", [P, NB * K], F32, kind="ExternalInput")
    t["cbp_d"] = nc.dram_tensor("cbp", [P, NB], F32, kind="ExternalInput")
    t["gbp_d"] = nc.dram_tensor("gbp", [P, NB], F32, kind="ExternalInput")
    t["rwp_d"] = nc.dram_tensor("rwp", [P, N * NB], F32, kind="ExternalInput")
    t["rwq_d"] = nc.dram_tensor("rwq", [P, N * NB], F32, kind="ExternalInput")
    t["abp_d"] = nc.dram_tensor("abp", [P, N * NB], F32, kind="ExternalInput")
    t["bbp_d"] = nc.dram_tensor("bbp", [P, N * NB], F32, kind="ExternalInput")
    t["ctbp_d"] = nc.dram_tensor("ctbp", [P, N * NB], F32, kind="ExternalInput")
    t["frwp_d"] = nc.dram_tensor("frwp", [P, NB], F32, kind="ExternalInput")
    t["fb1_d"] = nc.dram_tensor("fb1", [1, D], BF16, kind="ExternalInput")
    t["gwm_d"] = nc.dram_tensor("gwm", [NB, P, NB * P], BF16,
                                kind="ExternalInput")
    t["aw8_d"] = nc.dram_tensor("aw8", [N, K2, P, 2 * D], F8,
                                kind="ExternalInput")
    t["bw16_d"] = nc.dram_tensor("bw16", [N, D, D], BF16, kind="ExternalInput")
    t["cw8_d"] = nc.dram_tensor("cw8", [N, K2, P, 2 * D], F8,
                                kind="ExternalInput")
    t["fw16_d"] = nc.dram_tensor("fw16", [D, D], BF16, kind="ExternalInput")
    t["y_d"] = nc.dram_tensor("y", [T, D], F32, kind="ExternalOutput")

    with tile.TileContext(nc) as tc:
        _emit(nc, tc, t, reps=reps, no_cc=no_cc)
    nc.compile()
    return nc


def _emit(nc, tc, t, reps=1, no_cc=False):
    import contextlib
    with contextlib.ExitStack() as est:
        aux = est.enter_context(tc.tile_pool(name="aux", bufs=1))
        state = est.enter_context(tc.tile_pool(name="state", bufs=1))  # ctx/out
        alp = est.enter_context(tc.tile_pool(name="alp", bufs=1))   # alphas f32
        cnp = est.enter_context(tc.tile_pool(name="cnp", bufs=1))   # cn bf16
        cqp = est.enter_context(tc.tile_pool(name="cqp", bufs=1))   # cn fp8
        sip = est.enter_context(tc.tile_pool(name="sip", bufs=1))   # ws/sin/h bf16
        fqp = est.enter_context(tc.tile_pool(name="fqp", bufs=1))   # fe fp8
        sqp = est.enter_context(tc.tile_pool(name="sqp", bufs=1))   # x^2 fp8
        xsp = est.enter_context(tc.tile_pool(name="xsp", bufs=1))   # gate-x bf16
        vwp = est.enter_context(tc.tile_pool(name="vwp", bufs=3))   # v stream
        wap = est.enter_context(tc.tile_pool(name="wap", bufs=1))   # fp8 A w
        wcp = est.enter_context(tc.tile_pool(name="wcp", bufs=2))   # fp8 C w
        wbp = est.enter_context(tc.tile_pool(name="wbp", bufs=1))   # bf16 B/fin w
        gwp = est.enter_context(tc.tile_pool(name="gwp", bufs=1))   # gate w slices
        tmp = est.enter_context(tc.tile_pool(name="tmp", bufs=6))   # [P,SUB]
        mmp = est.enter_context(tc.tile_pool(name="mmp", bufs=6, space="PSUM"))
        ssp = est.enter_context(tc.tile_pool(name="ssp", bufs=1, space="PSUM"))
        bcp = est.enter_context(tc.tile_pool(name="bcp", bufs=1, space="PSUM"))
        dram = est.enter_context(tc.tile_pool(name="dram", bufs=1, space="DRAM"))

        def aux_load(name, shape, dt=F32):
            tl = aux.tile(shape, dt, name=name)
            nc.sync.dma_start(tl[:], t[name + "_d"][:])
            return tl

        mask = aux_load("mask", [P, 1])
        cwp = aux_load("cwp", [P, NB * K])
        cbp = aux_load("cbp", [P, NB])
        gbp = aux_load("gbp", [P, NB])
        rwp = aux_load("rwp", [P, N * NB])
        rwq = aux_load("rwq", [P, N * NB])
        abp = aux_load("abp", [P, N * NB])
        bbp = aux_load("bbp", [P, N * NB])
        ctbp = aux_load("ctbp", [P, N * NB])
        frwp = aux_load("frwp", [P, NB])
        fb1 = aux_load("fb1", [1, D], BF16)

        ones_f = aux.tile([P, 1], F32)
        nc.vector.memset(ones_f[:], 1.0)
        sixteen = aux.tile([P, 1], F32)
        nc.vector.memset(sixteen[:], 16.0)
        eps1 = aux.tile([1, 1], F32)
        nc.vector.memset(eps1[:], EPS)
        ones1_f = aux.tile([1, P], F32)
        nc.vector.memset(ones1_f[:], 1.0)
        ones1_r = aux.tile([1, P], F32R)
        nc.vector.tensor_copy(ones1_r[:], ones1_f[:])
        ones1_b = aux.tile([1, P], BF16)
        nc.vector.tensor_copy(ones1_b[:], ones1_f[:])
        ones_b = aux.tile([P, 1], BF16)
        nc.vector.tensor_copy(ones_b[:], ones_f[:])

        og_s = [dram.tile([P, T], BF16, name=f"og_s{nb}") for nb in range(NB)]
        v_s = [dram.tile([P, T], BF16, name=f"v_s{nb}") for nb in range(NB)]

        ctxb = [state.tile([P, T], BF16, name=f"ctx{nb}") for nb in range(NB)]
        outb = [state.tile([P, T], BF16, name=f"out{nb}") for nb in range(NB)]

        def rms_bc(src_pool_tag, srcs, tag_suffix):
            """Square(srcs) -> bf16; ones-matmul sum; Rsqrt [1,SUB]; bc.

            Returns list of bc PSUM tiles (per s)."""
            bcs = []
            for s in range(NS):
                sl = slice(s * SUB, (s + 1) * SUB)
                ssps = ssp.tile([1, SUB], F32, tag="ss", name=f"ss{tag_suffix}_{s}")
                for nb in range(NB):
                    sq = sqp.tile([P, SUB], BF16, tag="sq", bufs=3,
                                  name=f"sq{tag_suffix}_{s}_{nb}")
                    nc.vector.tensor_tensor(sq[:], srcs[nb][:, sl],
                                            srcs[nb][:, sl], OP.mult)
                    nc.tensor.matmul(ssps[:], ones_b[:], sq[:],
                                     start=(nb == 0), stop=(nb == NB - 1))
                sd1 = aux.tile([1, SUB], F32, tag="sd1",
                               name=f"sd{tag_suffix}_{s}", bufs=2)
                nc.scalar.activation(sd1[:], ssps[:], AF.Sqrt,
                                     bias=eps1[:, 0:1], scale=1.0 / D)
                invr = aux.tile([1, SUB], F32R, tag="invr",
                                name=f"invr{tag_suffix}_{s}", bufs=2)
                with nc.allow_low_precision(reason="bcast operand, f32r read"):
                    nc.vector.reciprocal(invr[:], sd1[:])
                bc = bcp.tile([P, SUB], F32, tag="bc", name=f"bc{tag_suffix}_{s}")
                nc.tensor.matmul(bc[:], ones1_r[:], invr[:], start=True,
                                 stop=True)
                bcs.append(bc)
            return bcs

        def one_pass(rep):
            # iteration-0 weights first: they gate the first A/B/C matmuls
            # and must not queue behind the bulk phase-0 streams
            def weights_load(i):
                wa = []
                for k2 in range(K2):
                    w = wap.tile([P, 2, D], F8, tag=f"wa{k2}",
                                 name=f"r{rep}_wa{i}_{k2}")
                    nc.sync.dma_start(
                        w[:], t["aw8_d"][i, k2].rearrange("p (j m) -> p j m", j=2))
                    wa.append(w)
                wb = []
                for k in range(NB):
                    w = wbp.tile([P, D], BF16, tag=f"wb{k}",
                                 name=f"r{rep}_wb{i}_{k}")
                    nc.sync.dma_start(w[:], t["bw16_d"][i, k * P:(k + 1) * P, :])
                    wb.append(w)
                wc = []
                for k2 in range(K2):
                    w = wcp.tile([P, 2, D], F8, tag=f"wc{k2}",
                                 name=f"r{rep}_wc{i}_{k2}")
                    nc.sync.dma_start(
                        w[:], t["cw8_d"][i, k2].rearrange("p (j m) -> p j m", j=2))
                    wc.append(w)
                return wa, wb, wc

            def gate_emit(i):
                # og[2i], og[2i+1] = silu(x @ gate_w + gbp); k-outer so gate-x
                # streams through 3 rotating slots
                gws = []
                for gi, m_g in enumerate((2 * i, 2 * i + 1)):
                    gw = gwp.tile([P, NB * P], BF16, tag=f"gw{gi}",
                                  name=f"r{rep}_gw{i}_{gi}")
                    nc.sync.dma_start(gw[:], t["gwm_d"][m_g])
                    gws.append(gw)
                gps = [[mmp.tile([P, SUB], F32, tag="mm",
                                 name=f"r{rep}_psg{i}_{gi}_{s}")
                        for s in range(NS)] for gi in range(2)]
                for k in range(NB):
                    xk = xsp.tile([P, T], BF16, tag=f"xs{k % 3}",
                                  name=f"r{rep}_xs{i}_{k}")
                    nc.sync.dma_start(xk[:], t["xtb_d"][k, :, K - 1:])
                    for gi in range(2):
                        for s in range(NS):
                            sl = slice(s * SUB, (s + 1) * SUB)
                            nc.tensor.matmul(
                                gps[gi][s][:], gws[gi][:, k * P:(k + 1) * P],
                                xk[:, sl],
                                start=(k == 0), stop=(k == NB - 1))
                for gi, m_g in enumerate((2 * i, 2 * i + 1)):
                    for s in range(NS):
                        sl = slice(s * SUB, (s + 1) * SUB)
                        ogt = tmp.tile([P, SUB], BF16, tag="bet",
                                       name=f"r{rep}_og{i}_{gi}_{s}", bufs=4)
                        nc.scalar.activation(ogt[:], gps[gi][s][:], AF.Silu,
                                             bias=gbp[:, m_g:m_g + 1])
                        nc.sync.dma_start(og_s[m_g][:, sl], ogt[:])

            # ---------- phase 0: load pre-transposed x, conv ----------
            xtb = [sip.tile([P, K - 1 + T], BF16, tag=f"sin{nb}",
                            name=f"r{rep}_xtb{nb}") for nb in range(NB)]
            for nb in range(NB):
                nc.sync.dma_start(xtb[nb][:], t["xtb_d"][nb])
            w_next = weights_load(0)
            # conv + v/ctx/out init
            for nb in range(NB):
                vt = vwp.tile([P, T], BF16, tag="vw", name=f"r{rep}_v0_{nb}")
                for s in range(NS):
                    sl = slice(s * SUB, (s + 1) * SUB)
                    cacc = tmp.tile([P, SUB], BF16, tag="bet", bufs=4,
                                    name=f"r{rep}_cacc{nb}_{s}")
                    nc.vector.tensor_scalar(
                        cacc[:], xtb[nb][:, s * SUB:s * SUB + SUB],
                        cwp[:, nb * K:nb * K + 1], None, OP.mult)
                    for k in range(1, K):
                        nc.vector.scalar_tensor_tensor(
                            cacc[:], xtb[nb][:, s * SUB + k:s * SUB + k + SUB],
                            cwp[:, nb * K + k:nb * K + k + 1],
                            cacc[:], OP.mult, OP.add)
                    nc.scalar.activation(vt[:, sl], cacc[:], AF.Silu,
                                         bias=cbp[:, nb:nb + 1])
                    nc.vector.tensor_copy(ctxb[nb][:, sl], vt[:, sl])
                    nc.vector.tensor_scalar(outb[nb][:, sl], vt[:, sl],
                                            S_FE, None, OP.mult)
                nc.sync.dma_start(v_s[nb][:], vt[:])

            gate_emit(0)

            # ---------- iterations ----------
            for i in range(N):
                wa, wb, wc = w_next
                if i + 1 < N:
                    w_next = weights_load(i + 1)
                vws = []
                for m in range(NB):
                    vw = vwp.tile([P, T], BF16, tag="vw", name=f"r{rep}_vw{i}_{m}")
                    nc.sync.dma_start(vw[:], v_s[m][:])
                    vws.append(vw)

                # rms -> bc[s]
                bcs = rms_bc("ctx", ctxb, f"r{i}")
                bcb = []
                for s in range(NS):
                    bb = aux.tile([P, SUB], BF16, tag="bcb", bufs=4,
                                  name=f"r{rep}_bcb{i}_{s}")
                    nc.vector.tensor_copy(bb[:], bcs[s][:])
                    bcb.append(bb)
                cn = [cnp.tile([P, T], BF16, tag=f"cn{nb}",
                               name=f"r{rep}_cn{i}_{nb}") for nb in range(NB)]
                cnq = [cqp.tile([P, 2, T], F8, tag=f"cq{k2}",
                                name=f"r{rep}_cnq{i}_{k2}") for k2 in range(K2)]
                for nb in range(NB):
                    k2, j = nb // 2, nb % 2
                    for s in range(NS):
                        sl = slice(s * SUB, (s + 1) * SUB)
                        nc.vector.scalar_tensor_tensor(
                            cn[nb][:, sl], ctxb[nb][:, sl],
                            rwp[:, i * NB + nb:i * NB + nb + 1], bcb[s][:],
                            OP.mult, OP.mult)
                        nc.vector.scalar_tensor_tensor(
                            cnq[k2][:, j, sl], ctxb[nb][:, sl],
                            rwq[:, i * NB + nb:i * NB + nb + 1], bcb[s][:],
                            OP.mult, OP.mult)

                # A: alphas = sigmoid(dq * (cnq @ wa) + abp)
                alphas = [alp.tile([P, T], F32, tag=f"al{nb}",
                                   name=f"r{rep}_alphas{i}_{nb}")
                          for nb in range(NB)]
                for m in range(NB):
                    for s in range(NS):
                        sl = slice(s * SUB, (s + 1) * SUB)
                        ps = mmp.tile([P, SUB], F32, tag="mm",
                                      name=f"r{rep}_psa{i}_{m}_{s}")
                        for k2 in range(K2):
                            nc.tensor.matmul(
                                ps[:], wa[k2][:, :, m * P:(m + 1) * P],
                                cnq[k2][:, :, sl],
                                start=(k2 == 0), stop=(k2 == K2 - 1),
                                perf_mode=PM.DoubleRow)
                        nc.scalar.activation(
                            alphas[m][:, sl], ps[:], AF.Sigmoid,
                            bias=abp[:, i * NB + m:i * NB + m + 1], scale=DQ_A)

                # ws' = sqrt(16 - 16 a^2) -> sin, then B per half so the
                # ACT queue (sqrt batch vs bet evacs) never starves B's PSUM
                sin = [sip.tile([P, T], BF16, tag=f"sin{nb}",
                                name=f"r{rep}_sin{i}_{nb}") for nb in range(NB)]
                carries = aux.tile([P, NB], F32, tag="carries", bufs=2,
                                   name=f"r{rep}_carries{i}")

                def ws_half(ms):
                    for m in ms:
                        for s in range(NS):
                            sl = slice(s * SUB, (s + 1) * SUB)
                            asq = tmp.tile([P, SUB], F32, tag="tmp",
                                           name=f"r{rep}_asq{i}_{m}_{s}")
                            nc.scalar.activation(asq[:], alphas[m][:, sl],
                                                 AF.Square)
                            nc.scalar.activation(sin[m][:, sl], asq[:], AF.Sqrt,
                                                 bias=sixteen[:, 0:1],
                                                 scale=-16.0)

                def b_half(ms):
                    for m in ms:
                        for s in range(NS):
                            sl = slice(s * SUB, (s + 1) * SUB)
                            ps = mmp.tile([P, SUB], F32, tag="mm",
                                          name=f"r{rep}_psb{i}_{m}_{s}")
                            for k in range(NB):
                                nc.tensor.matmul(
                                    ps[:], wb[k][:, m * P:(m + 1) * P],
                                    cn[k][:, sl],
                                    start=(k == 0), stop=(k == NB - 1))
                            bet = tmp.tile([P, SUB], BF16, tag="bet",
                                           name=f"r{rep}_bet{i}_{m}_{s}", bufs=4)
                            nc.scalar.activation(
                                bet[:], ps[:], AF.Silu,
                                bias=bbp[:, i * NB + m:i * NB + m + 1])
                            nc.vector.tensor_tensor(bet[:], bet[:],
                                                    vws[m][:, sl], OP.mult)
                            nc.vector.tensor_tensor(sin[m][:, sl], sin[m][:, sl],
                                                    bet[:], OP.mult)
                        nc.vector.tensor_tensor_scan(
                            sin[m][:], alphas[m][:], sin[m][:], 0.0,
                            OP.mult, OP.add)
                        nc.vector.tensor_copy(carries[:, m:m + 1],
                                              sin[m][:, T - 1:T])
                        nc.vector.tensor_tensor_scan(
                            alphas[m][:], alphas[m][:], alphas[m][:],
                            1.0, OP.mult, OP.bypass)

                HB = NB // 2

                def carry_exchange(half, col0):
                    hd = D // 2
                    cin = dram.tile([hd], F32, name=f"r{rep}_cin{i}_{half}")
                    cout = dram.tile([2, hd], F32, name=f"r{rep}_cout{i}_{half}")
                    nc.sync.dma_start(
                        cin[:].rearrange("(p nb) -> p nb", p=P),
                        carries[:, col0:col0 + HB])
                    return cin, cout

                def carry_recv(half, cin, cout):
                    if no_cc:
                        nc.sync.dma_start(
                            cout[0:1, :], cin[:].rearrange("(a b) -> a b", a=1))
                        nc.sync.dma_start(
                            cout[1:2, :], cin[:].rearrange("(a b) -> a b", a=1))
                    else:
                        nc.gpsimd.collective_compute(
                            "AllGather", OP.bypass,
                            replica_groups=[[0, 1], [2, 3], [4, 5], [6, 7]],
                            ins=[cin.opt()], outs=[cout.opt()])
                    gsb = aux.tile([P, HB], F32, tag=f"gsb{half}", bufs=2,
                                   name=f"r{rep}_gsb{i}_{half}")
                    nc.sync.dma_start(
                        gsb[:],
                        cout[0:1, :].rearrange("a (p nb) -> (a p) nb", p=P))
                    ceff = aux.tile([P, HB], F32, tag=f"ceff{half}", bufs=2,
                                    name=f"r{rep}_ceff{i}_{half}")
                    nc.vector.tensor_scalar(ceff[:], gsb[:], mask[:, 0:1],
                                            None, OP.mult)
                    return ceff

                ws_half(range(0, HB))
                b_half(range(0, HB))
                cin_a, cout_a = carry_exchange(0, 0)
                ceff_a = carry_recv(0, cin_a, cout_a)
                ws_half(range(HB, NB))
                b_half(range(HB, NB))
                cin_b, cout_b = carry_exchange(1, HB)

                # deferred gate (fills the AG_b window with PE work);
                # iteration 0's gate already ran in the phase-0 conv hole
                if i >= 1:
                    gate_emit(i)

                ceff_b = carry_recv(1, cin_b, cout_b)

                # fe = cumprod*ceff + h (in place over sin); fe8; out +=
                feq = [fqp.tile([P, 2, T], F8, tag=f"fq{k2}",
                                name=f"r{rep}_feq{i}_{k2}") for k2 in range(K2)]
                for nb in range(NB):
                    k2, j = nb // 2, nb % 2
                    ceff_h = ceff_a if nb < HB else ceff_b
                    nc.vector.scalar_tensor_tensor(
                        sin[nb][:], alphas[nb][:], ceff_h[:, nb % HB:nb % HB + 1],
                        sin[nb][:], OP.mult, OP.add)
                    nc.vector.tensor_copy(feq[k2][:, j, :], sin[nb][:])
                    nc.gpsimd.tensor_tensor(outb[nb][:], outb[nb][:],
                                            sin[nb][:], OP.add)

                # C: ctx += silu(dq * (feq @ wc) + ctbp); s-outer so the
                # next iteration's rms(s0) can start while C(s1) runs
                for s in range(NS):
                    for m in range(NB):
                        sl = slice(s * SUB, (s + 1) * SUB)
                        ps = mmp.tile([P, SUB], F32, tag="mm",
                                      name=f"r{rep}_psc{i}_{m}_{s}")
                        for k2 in range(K2):
                            nc.tensor.matmul(
                                ps[:], wc[k2][:, :, m * P:(m + 1) * P],
                                feq[k2][:, :, sl],
                                start=(k2 == 0), stop=(k2 == K2 - 1),
                                perf_mode=PM.DoubleRow)
                        cu = tmp.tile([P, SUB], BF16, tag="bet",
                                      name=f"r{rep}_cu{i}_{m}_{s}", bufs=4)
                        nc.scalar.activation(
                            cu[:], ps[:], AF.Silu,
                            bias=ctbp[:, i * NB + m:i * NB + m + 1], scale=DQ_C)
                        nc.gpsimd.tensor_tensor(ctxb[m][:, sl], ctxb[m][:, sl],
                                                cu[:], OP.add)

            # ---------- final ----------
            ogl = [sip.tile([P, T], BF16, tag=f"sin{nb}", name=f"r{rep}_ogl{nb}")
                   for nb in range(NB)]
            for nb in range(NB):
                nc.sync.dma_start(ogl[nb][:], og_s[nb][:])
                nc.vector.tensor_tensor(outb[nb][:], outb[nb][:], ogl[nb][:],
                                        OP.mult)
            bcs = rms_bc("fin", outb, "rf")
            bcb = []
            for s in range(NS):
                bb = aux.tile([P, SUB], BF16, tag="bcb", bufs=4,
                              name=f"r{rep}_bcbf{s}")
                nc.vector.tensor_copy(bb[:], bcs[s][:])
                bcb.append(bb)
            fo = [cnp.tile([P, T], BF16, tag=f"cn{nb}", name=f"r{rep}_fo{nb}")
                  for nb in range(NB)]
            for nb in range(NB):
                for s in range(NS):
                    sl = slice(s * SUB, (s + 1) * SUB)
                    nc.vector.scalar_tensor_tensor(
                        fo[nb][:, sl], outb[nb][:, sl],
                        frwp[:, nb:nb + 1], bcb[s][:], OP.mult, OP.mult)
            fw = []
            for k in range(NB):
                w = wbp.tile([P, D], BF16, tag=f"wb{k}", name=f"r{rep}_fw{k}")
                nc.sync.dma_start(w[:], t["fw16_d"][k * P:(k + 1) * P, :])
                fw.append(w)
            for tb in range(NB):
                for do in range(NS):
                    ps = mmp.tile([P, SUB], F32, tag="mm",
                                  name=f"r{rep}_psf{tb}_{do}")
                    for k in range(NB):
                        nc.tensor.matmul(ps[:], fo[k][:, tb * P:(tb + 1) * P],
                                         fw[k][:, do * SUB:(do + 1) * SUB],
                                         start=(k == 0), stop=False)
                    nc.tensor.matmul(ps[:], ones1_b[:],
                                     fb1[:, do * SUB:(do + 1) * SUB],
                                     start=False, stop=True)
                    yt = tmp.tile([P, SUB], F32, tag="tmp",
                                  name=f"r{rep}_yt{tb}_{do}")
                    nc.scalar.activation(yt[:], ps[:], AF.Silu)
                    nc.sync.dma_start(
                        t["y_d"][tb * P:(tb + 1) * P, do * SUB:(do + 1) * SUB],
                        yt[:])

        for rep in range(reps):
            one_pass(rep)


def _q8(a, scale):
    return np.clip(np.asarray(a, np.float32) * scale, -240.0, 240.0).astype(E4NP)


def _prep_in_maps(inputs):
    f32 = lambda k: np.asarray(inputs[k], np.float32)
    x = f32("x")
    conv_w, conv_b = f32("conv_w"), f32("conv_b")
    gate_w, gate_b = f32("gate_w"), f32("gate_b")
    rms_w = f32("rms_w")
    alpha_w, alpha_b = f32("alpha_w"), f32("alpha_b")
    beta_w, beta_b = f32("beta_w"), f32("beta_b")
    ctx_w, ctx_b = f32("ctx_w"), f32("ctx_b")
    fin_rms_w, fin_w, fin_b = f32("fin_rms_w"), f32("fin_w"), f32("fin_b")

    def pack1(a):
        return np.ascontiguousarray(a.reshape(NB, P).T)

    def packN(a):
        return np.ascontiguousarray(
            a.reshape(N, NB, P).transpose(2, 0, 1).reshape(P, N * NB))

    def pack_dr(w, scale):      # [N, D, D] -> [N, K2, P, 2*D] fp8
        wq = _q8(w, scale)      # e4m3
        return np.ascontiguousarray(
            wq.reshape(N, K2, 2, P, D).transpose(0, 1, 3, 2, 4)
            .reshape(N, K2, P, 2 * D))

    cwp = np.ascontiguousarray(
        conv_w.T.reshape(NB, P, K).transpose(1, 0, 2).reshape(P, NB * K))
    gwm = np.ascontiguousarray(
        gate_w.reshape(NB, P, NB, P).transpose(2, 1, 0, 3)
        .reshape(NB, P, NB * P)).astype(BFNP)

    shared = dict(
        cwp=cwp, cbp=pack1(conv_b), gbp=pack1(gate_b),
        rwp=packN(rms_w), rwq=packN(rms_w * S_CN),
        abp=packN(alpha_b), bbp=packN(beta_b), ctbp=packN(ctx_b),
        frwp=pack1(fin_rms_w),
        fb1=np.ascontiguousarray(fin_b[None, :]).astype(BFNP),
        gwm=gwm,
        aw8=pack_dr(alpha_w, S_AW),
        bw16=np.ascontiguousarray(beta_w).astype(BFNP),
        cw8=pack_dr(ctx_w, S_CW),
        fw16=np.ascontiguousarray(fin_w).astype(BFNP),
    )
    in_maps = []
    for c in range(8):
        b, h = c // 2, c % 2
        t0 = h * T
        m = dict(shared)
        xt = x[b, t0:t0 + T].T.reshape(NB, P, T)         # [NB, P, T]
        if h == 0:
            halo = np.zeros((NB, P, K - 1), np.float32)
        else:
            halo = np.ascontiguousarray(
                x[b, t0 - (K - 1):t0, :].T.reshape(NB, P, K - 1))
        m["xtb"] = np.concatenate([halo, xt], axis=2).astype(BFNP)
        m["mask"] = np.full((P, 1), float(h), np.float32)
        in_maps.append(m)
    return in_maps


def kernel(**inputs) -> np.ndarray:
    if "nc" not in _CACHE:
        _CACHE["nc"] = _build()
    nc = _CACHE["nc"]
    in_maps = _prep_in_maps(inputs)
    res = bass_utils.run_bass_kernel_spmd(nc, in_maps, core_ids=list(range(8)))
    y = np.empty((B, S, D), np.float32)
    for c in range(8):
        b, h = c // 2, c % 2
        y[b, h * T:(h + 1) * T] = res.results[c]["y"]
    return y
